# revision 4
# baseline (speedup 1.0000x reference)
"""Trainium2 Bass kernel for nn_CrossEntropyLoss_22419729285187.

Computes  -sum_{matched, non-BG true rows} dot(y_true[i,1:], y_pred[rank_i]) / count
sharded over 8 NeuronCores.

Strategy (per sharding hint): shard y_true rows (N) across the 8 cores.
The host performs the cheap key join (encode + searchsorted + cumsum) to
produce, per true row, the positionally-aligned y_pred row and a validity
mask; the device streams the full feature payload (y_true_features shard,
aligned y_pred features, mask) and does the fused multiply-reduce +
count, emitting per-partition partial sums. Host combines 8x[128,2]
partials into the final scalar.
"""

import os
import sys

for _p in ("/opt/trn_rl_repo", "/root/.axon_site/_ro/trn_rl_repo"):
    if os.path.isdir(_p) and _p not in sys.path:
        sys.path.append(_p)

import numpy as np

N_CORES = 8

# Device-side tiling: rows are laid out [tile t][partition p][group g];
# each of the 128 partitions owns G consecutive rows per tile.
PARTS = 128
G = 32  # rows per partition per tile

_compiled = {}
_last_results = None


def _encode(idx):
    idx = idx.astype(np.int64)
    return ((idx[:, 0] * 1024 + idx[:, 1]) * 1024 + idx[:, 2]) * 1024 + idx[:, 3]


def _build_program(r_pad, c_pred):
    """Build + schedule the SPMD Tile program for one core shard of
    r_pad rows (multiple of PARTS*G). yt carries only feature cols 1:33
    (the BG column is folded into ypal/aux on the host)."""
    from concourse import bacc
    import concourse.mybir as mybir
    from concourse.tile import TileContext

    f32 = mybir.dt.float32
    n_tiles = r_pad // (PARTS * G)

    nc = bacc.Bacc("TRN2", target_bir_lowering=False, debug=False,
                   num_devices=N_CORES)
    yt_d = nc.dram_tensor("yt", [r_pad, c_pred], f32, kind="ExternalInput")
    yp_d = nc.dram_tensor("ypal", [r_pad, c_pred], f32, kind="ExternalInput")
    ax_d = nc.dram_tensor("aux", [r_pad // G, G], f32, kind="ExternalInput")
    out_d = nc.dram_tensor("partials", [PARTS, 2], f32, kind="ExternalOutput")

    yt_v = yt_d.ap().rearrange("(t p g) c -> t p (g c)", p=PARTS, g=G)
    yp_v = yp_d.ap().rearrange("(t p g) c -> t p (g c)", p=PARTS, g=G)
    ax_v = ax_d.ap().rearrange("(t p) g -> t p g", p=PARTS)

    with TileContext(nc) as tc:
        with tc.tile_pool(name="acc", bufs=1) as accp:
            num_acc = accp.tile([PARTS, 1], f32)
            k_acc = accp.tile([PARTS, 1], f32)
            nc.vector.memset(num_acc[:], 0.0)
            nc.vector.memset(k_acc[:], 0.0)
            with tc.tile_pool(name="io", bufs=4) as pool:
                for t in range(n_tiles):
                    yt_t = pool.tile([PARTS, G * c_pred], f32, tag="yt")
                    yp_t = pool.tile([PARTS, G * c_pred], f32, tag="yp")
                    ax_t = pool.tile([PARTS, G], f32, tag="ax")
                    nc.sync.dma_start(out=yt_t[:], in_=yt_v[t])
                    nc.sync.dma_start(out=yp_t[:], in_=yp_v[t])
                    nc.sync.dma_start(out=ax_t[:], in_=ax_v[t])
                    scr = pool.tile([PARTS, G * c_pred], f32, tag="scr")
                    red = pool.tile([PARTS, 1], f32, tag="red")
                    kred = pool.tile([PARTS, 1], f32, tag="kred")
                    # num_acc += sum_{g,c} yt[:, g, c] * ypal[:, g, c]
                    nc.vector.tensor_mul(out=scr[:], in0=yt_t[:], in1=yp_t[:])
                    nc.vector.tensor_reduce(out=red[:], in_=scr[:],
                                            axis=mybir.AxisListType.X,
                                            op=mybir.AluOpType.add)
                    nc.vector.tensor_add(out=num_acc[:], in0=num_acc[:],
                                         in1=red[:])
                    # k_acc += sum_g mask
                    nc.vector.tensor_reduce(out=kred[:], in_=ax_t[:],
                                            axis=mybir.AxisListType.X,
                                            op=mybir.AluOpType.add)
                    nc.vector.tensor_add(out=k_acc[:], in0=k_acc[:],
                                         in1=kred[:])
            nc.sync.dma_start(out=out_d[:, 0:1], in_=num_acc[:])
            nc.sync.dma_start(out=out_d[:, 1:2], in_=k_acc[:])
    nc.compile()
    return nc


def kernel(y_true_features, y_true_indices, y_pred_features, y_pred_indices):
    global _last_results
    from concourse.bass_utils import run_bass_kernel_spmd

    yt = np.ascontiguousarray(np.asarray(y_true_features, dtype=np.float32))
    yp = np.ascontiguousarray(np.asarray(y_pred_features, dtype=np.float32))
    n, c1 = yt.shape
    m, c = yp.shape

    # ---- host-side key join (cheap integer work) ----
    kt = _encode(np.asarray(y_true_indices))
    kp = _encode(np.asarray(y_pred_indices))
    kps = np.sort(kp)
    pos = np.clip(np.searchsorted(kps, kt), 0, m - 1)
    matched = kps[pos] == kt
    rank = np.cumsum(matched, dtype=np.int64) - 1
    clip_rank = np.clip(rank, 0, m - 1)
    notbg = matched & (yt[:, 0] != 1.0)

    # positionally aligned pred rows, zeroed where the row doesn't count
    ypal = yp[clip_rank]
    ypal[~notbg] = 0.0
    aux = notbg.astype(np.float32)

    # ---- shard across cores ----
    assert n % N_CORES == 0
    rows = n // N_CORES
    r_pad = -(-rows // (PARTS * G)) * (PARTS * G)

    key = (r_pad, c)
    if key not in _compiled:
        _compiled[key] = _build_program(r_pad, c)
    nc = _compiled[key]

    in_maps = []
    for i in range(N_CORES):
        lo, hi = i * rows, (i + 1) * rows
        yt_c = np.zeros((r_pad, c), dtype=np.float32)
        yt_c[:rows] = yt[lo:hi, 1:]
        yp_c = np.zeros((r_pad, c), dtype=np.float32)
        yp_c[:rows] = ypal[lo:hi]
        ax_c = np.zeros((r_pad,), dtype=np.float32)
        ax_c[:rows] = aux[lo:hi]
        in_maps.append({
            "yt": yt_c,
            "ypal": yp_c,
            "aux": ax_c.reshape(r_pad // G, G),
        })

    res = run_bass_kernel_spmd(nc, in_maps, list(range(N_CORES)))
    _last_results = res

    num = 0.0
    k = 0.0
    for i in range(N_CORES):
        p = res.results[i]["partials"]
        num += float(p[:, 0].sum(dtype=np.float64))
        k += float(p[:, 1].sum(dtype=np.float64))
    return np.float32(-num / k)


# revision 5
# speedup vs baseline: 1.2601x; 1.2601x over previous
"""Trainium2 Bass kernel for nn_CrossEntropyLoss_22419729285187.

Computes  -sum_{matched, non-BG true rows} dot(y_true[i,1:], y_pred[rank_i]) / count
sharded over 8 NeuronCores.

Strategy (per sharding hint): shard y_true rows (N) across the 8 cores.
The host performs the cheap key join (encode + searchsorted + cumsum) to
produce, per true row, the positionally-aligned y_pred row and a validity
mask; the device streams the full feature payload (y_true_features shard,
aligned y_pred features, mask) and does the fused multiply-reduce +
count, emitting per-partition partial sums. Host combines 8x[128,2]
partials into the final scalar.
"""

import os
import sys

for _p in ("/opt/trn_rl_repo", "/root/.axon_site/_ro/trn_rl_repo"):
    if os.path.isdir(_p) and _p not in sys.path:
        sys.path.append(_p)

import numpy as np

N_CORES = 8

# Device-side tiling: rows are laid out [tile t][partition p][group g];
# each of the 128 partitions owns G consecutive rows per tile.
PARTS = 128
G = 32  # rows per partition per tile

_compiled = {}
_last_results = None


def _encode(idx):
    idx = idx.astype(np.int64)
    return ((idx[:, 0] * 1024 + idx[:, 1]) * 1024 + idx[:, 2]) * 1024 + idx[:, 3]


def _build_program(r_pad, c_pred):
    """Build + schedule the SPMD Tile program for one core shard of
    r_pad rows (multiple of PARTS*G). yt carries only feature cols 1:33
    (the BG column is folded into ypal/aux on the host)."""
    from concourse import bacc
    import concourse.mybir as mybir
    from concourse.tile import TileContext

    f32 = mybir.dt.float32
    n_tiles = r_pad // (PARTS * G)

    nc = bacc.Bacc("TRN2", target_bir_lowering=False, debug=False,
                   num_devices=N_CORES)
    yt_d = nc.dram_tensor("yt", [r_pad, c_pred], f32, kind="ExternalInput")
    yp_d = nc.dram_tensor("ypal", [r_pad, c_pred], f32, kind="ExternalInput")
    ax_d = nc.dram_tensor("aux", [r_pad // G, G], f32, kind="ExternalInput")
    out_d = nc.dram_tensor("partials", [PARTS, 2], f32, kind="ExternalOutput")

    yt_v = yt_d.ap().rearrange("(t p g) c -> t p (g c)", p=PARTS, g=G)
    yp_v = yp_d.ap().rearrange("(t p g) c -> t p (g c)", p=PARTS, g=G)
    ax_v = ax_d.ap().rearrange("(t p) g -> t p g", p=PARTS)

    with TileContext(nc) as tc:
        with tc.tile_pool(name="acc", bufs=1) as accp:
            num_acc = accp.tile([PARTS, 1], f32)
            k_acc = accp.tile([PARTS, 1], f32)
            nc.vector.memset(num_acc[:], 0.0)
            nc.vector.memset(k_acc[:], 0.0)
            with tc.tile_pool(name="io", bufs=4) as pool:
                for t in range(n_tiles):
                    yt_t = pool.tile([PARTS, G * c_pred], f32, tag="yt")
                    yp_t = pool.tile([PARTS, G * c_pred], f32, tag="yp")
                    ax_t = pool.tile([PARTS, G], f32, tag="ax")
                    nc.sync.dma_start(out=yt_t[:], in_=yt_v[t])
                    nc.sync.dma_start(out=yp_t[:], in_=yp_v[t])
                    nc.sync.dma_start(out=ax_t[:], in_=ax_v[t])
                    scr = pool.tile([PARTS, G * c_pred], f32, tag="scr")
                    red = pool.tile([PARTS, 1], f32, tag="red")
                    kred = pool.tile([PARTS, 1], f32, tag="kred")
                    # num_acc += sum_{g,c} yt[:, g, c] * ypal[:, g, c]
                    nc.vector.tensor_mul(out=scr[:], in0=yt_t[:], in1=yp_t[:])
                    nc.vector.tensor_reduce(out=red[:], in_=scr[:],
                                            axis=mybir.AxisListType.X,
                                            op=mybir.AluOpType.add)
                    nc.vector.tensor_add(out=num_acc[:], in0=num_acc[:],
                                         in1=red[:])
                    # k_acc += sum_g mask
                    nc.vector.tensor_reduce(out=kred[:], in_=ax_t[:],
                                            axis=mybir.AxisListType.X,
                                            op=mybir.AluOpType.add)
                    nc.vector.tensor_add(out=k_acc[:], in0=k_acc[:],
                                         in1=kred[:])
            nc.sync.dma_start(out=out_d[:, 0:1], in_=num_acc[:])
            nc.sync.dma_start(out=out_d[:, 1:2], in_=k_acc[:])
    nc.compile()
    return nc


def kernel(y_true_features, y_true_indices, y_pred_features, y_pred_indices):
    global _last_results
    from concourse.bass_utils import run_bass_kernel_spmd

    yt = np.ascontiguousarray(np.asarray(y_true_features, dtype=np.float32))
    yp = np.ascontiguousarray(np.asarray(y_pred_features, dtype=np.float32))
    n, c1 = yt.shape
    m, c = yp.shape

    # ---- host-side key join (cheap integer work) ----
    kt = _encode(np.asarray(y_true_indices))
    kp = _encode(np.asarray(y_pred_indices))
    kps = np.sort(kp)
    pos = np.clip(np.searchsorted(kps, kt), 0, m - 1)
    matched = kps[pos] == kt
    # Only matched true rows contribute to num and k. The r-th matched
    # true row (row order) pairs with y_pred_features[r] positionally
    # (rank = cumsum(matched)-1 is sequential over matched rows), so the
    # pred side needs no gather at all — just the first m_eff rows.
    midx = np.flatnonzero(matched)
    m_eff = midx.size
    yt_cmp = yt[midx, 1:]                      # [m_eff, c] gather
    notbg = yt[midx, 0] != 1.0
    yt_cmp[~notbg] = 0.0                       # BG pairs contribute 0
    aux = notbg.astype(np.float32)

    # ---- shard the m_eff matched pairs across cores ----
    rows = -(-m_eff // N_CORES)
    r_pad = -(-rows // (PARTS * G)) * (PARTS * G)

    key = (r_pad, c)
    if key not in _compiled:
        _compiled[key] = _build_program(r_pad, c)
    nc = _compiled[key]

    in_maps = []
    for i in range(N_CORES):
        lo, hi = i * rows, min((i + 1) * rows, m_eff)
        nr = max(hi - lo, 0)
        yt_c = np.zeros((r_pad, c), dtype=np.float32)
        yt_c[:nr] = yt_cmp[lo:hi]
        yp_c = np.zeros((r_pad, c), dtype=np.float32)
        yp_c[:nr] = yp[lo:hi]
        ax_c = np.zeros((r_pad,), dtype=np.float32)
        ax_c[:nr] = aux[lo:hi]
        in_maps.append({
            "yt": yt_c,
            "ypal": yp_c,
            "aux": ax_c.reshape(r_pad // G, G),
        })

    res = run_bass_kernel_spmd(nc, in_maps, list(range(N_CORES)))
    _last_results = res

    num = 0.0
    k = 0.0
    for i in range(N_CORES):
        p = res.results[i]["partials"]
        num += float(p[:, 0].sum(dtype=np.float64))
        k += float(p[:, 1].sum(dtype=np.float64))
    return np.float32(-num / k)


# revision 6
# speedup vs baseline: 1.5562x; 1.2350x over previous
"""Trainium2 Bass kernel for nn_CrossEntropyLoss_22419729285187.

Computes  -sum_{matched, non-BG true rows} dot(y_true[i,1:], y_pred[rank_i]) / count
sharded over 8 NeuronCores.

Strategy (per sharding hint): shard y_true rows (N) across the 8 cores.
The host performs the cheap key join (encode + searchsorted + cumsum) to
produce, per true row, the positionally-aligned y_pred row and a validity
mask; the device streams the full feature payload (y_true_features shard,
aligned y_pred features, mask) and does the fused multiply-reduce +
count, emitting per-partition partial sums. Host combines 8x[128,2]
partials into the final scalar.
"""

import os
import sys

for _p in ("/opt/trn_rl_repo", "/root/.axon_site/_ro/trn_rl_repo"):
    if os.path.isdir(_p) and _p not in sys.path:
        sys.path.append(_p)

import numpy as np

N_CORES = 8

# Device-side tiling: rows are laid out [tile t][partition p][group g];
# each of the 128 partitions owns G consecutive rows per tile.
PARTS = 128
G = 32  # rows per partition per tile

_compiled = {}
_last_results = None


def _encode(idx):
    idx = idx.astype(np.int64)
    return ((idx[:, 0] * 1024 + idx[:, 1]) * 1024 + idx[:, 2]) * 1024 + idx[:, 3]


def _build_program(r_pad, c_pred):
    """Build + schedule the SPMD Tile program for one core shard of
    r_pad rows (multiple of PARTS*G). yt carries only feature cols 1:33
    (the BG column is folded into ypal/aux on the host)."""
    from concourse import bacc
    import concourse.mybir as mybir
    from concourse.tile import TileContext

    f32 = mybir.dt.float32
    n_tiles = r_pad // (PARTS * G)

    nc = bacc.Bacc("TRN2", target_bir_lowering=False, debug=False,
                   num_devices=N_CORES)
    yt_d = nc.dram_tensor("yt", [r_pad, c_pred], f32, kind="ExternalInput")
    yp_d = nc.dram_tensor("ypal", [r_pad, c_pred], f32, kind="ExternalInput")
    ax_d = nc.dram_tensor("aux", [r_pad // G, G], f32, kind="ExternalInput")
    out_d = nc.dram_tensor("partials", [PARTS, 2], f32, kind="ExternalOutput")

    yt_v = yt_d.ap().rearrange("(t p g) c -> t p (g c)", p=PARTS, g=G)
    yp_v = yp_d.ap().rearrange("(t p g) c -> t p (g c)", p=PARTS, g=G)
    ax_v = ax_d.ap().rearrange("(t p) g -> t p g", p=PARTS)

    with TileContext(nc) as tc:
        with tc.tile_pool(name="acc", bufs=1) as accp:
            # per-tile partial sums land in their own column; reduced once
            red_all = accp.tile([PARTS, n_tiles], f32)
            k_all = accp.tile([PARTS, n_tiles], f32)
            num_acc = accp.tile([PARTS, 1], f32)
            k_acc = accp.tile([PARTS, 1], f32)
            with tc.tile_pool(name="io", bufs=4) as pool:
                for t in range(n_tiles):
                    yt_t = pool.tile([PARTS, G * c_pred], f32, tag="yt")
                    yp_t = pool.tile([PARTS, G * c_pred], f32, tag="yp")
                    ax_t = pool.tile([PARTS, G], f32, tag="ax")
                    nc.sync.dma_start(out=yt_t[:], in_=yt_v[t])
                    nc.scalar.dma_start(out=yp_t[:], in_=yp_v[t])
                    nc.gpsimd.dma_start(out=ax_t[:], in_=ax_v[t])
                    scr = pool.tile([PARTS, G * c_pred], f32, tag="scr")
                    kscr = pool.tile([PARTS, G], f32, tag="kscr")
                    # red_all[:, t] = sum_{g,c} yt[:, g, c] * ypal[:, g, c]
                    nc.vector.scalar_tensor_tensor(
                        out=scr[:], in0=yt_t[:], scalar=1.0, in1=yp_t[:],
                        op0=mybir.AluOpType.mult, op1=mybir.AluOpType.mult,
                        accum_out=red_all[:, t:t + 1])
                    # k_all[:, t] = sum_g mask
                    nc.vector.scalar_tensor_tensor(
                        out=kscr[:], in0=ax_t[:], scalar=1.0, in1=ax_t[:],
                        op0=mybir.AluOpType.mult, op1=mybir.AluOpType.mult,
                        accum_out=k_all[:, t:t + 1])
            nc.vector.tensor_reduce(out=num_acc[:], in_=red_all[:],
                                    axis=mybir.AxisListType.X,
                                    op=mybir.AluOpType.add)
            nc.vector.tensor_reduce(out=k_acc[:], in_=k_all[:],
                                    axis=mybir.AxisListType.X,
                                    op=mybir.AluOpType.add)
            nc.sync.dma_start(out=out_d[:, 0:1], in_=num_acc[:])
            nc.sync.dma_start(out=out_d[:, 1:2], in_=k_acc[:])
    nc.compile()
    return nc


def kernel(y_true_features, y_true_indices, y_pred_features, y_pred_indices):
    global _last_results
    from concourse.bass_utils import run_bass_kernel_spmd

    yt = np.ascontiguousarray(np.asarray(y_true_features, dtype=np.float32))
    yp = np.ascontiguousarray(np.asarray(y_pred_features, dtype=np.float32))
    n, c1 = yt.shape
    m, c = yp.shape

    # ---- host-side key join (cheap integer work) ----
    kt = _encode(np.asarray(y_true_indices))
    kp = _encode(np.asarray(y_pred_indices))
    kps = np.sort(kp)
    pos = np.clip(np.searchsorted(kps, kt), 0, m - 1)
    matched = kps[pos] == kt
    # Only matched true rows contribute to num and k. The r-th matched
    # true row (row order) pairs with y_pred_features[r] positionally
    # (rank = cumsum(matched)-1 is sequential over matched rows), so the
    # pred side needs no gather at all — just the first m_eff rows.
    midx = np.flatnonzero(matched)
    m_eff = midx.size
    yt_cmp = yt[midx, 1:]                      # [m_eff, c] gather
    notbg = yt[midx, 0] != 1.0
    yt_cmp[~notbg] = 0.0                       # BG pairs contribute 0
    aux = notbg.astype(np.float32)

    # ---- shard the m_eff matched pairs across cores ----
    rows = -(-m_eff // N_CORES)
    r_pad = -(-rows // (PARTS * G)) * (PARTS * G)

    key = (r_pad, c)
    if key not in _compiled:
        _compiled[key] = _build_program(r_pad, c)
    nc = _compiled[key]

    in_maps = []
    for i in range(N_CORES):
        lo, hi = i * rows, min((i + 1) * rows, m_eff)
        nr = max(hi - lo, 0)
        yt_c = np.zeros((r_pad, c), dtype=np.float32)
        yt_c[:nr] = yt_cmp[lo:hi]
        yp_c = np.zeros((r_pad, c), dtype=np.float32)
        yp_c[:nr] = yp[lo:hi]
        ax_c = np.zeros((r_pad,), dtype=np.float32)
        ax_c[:nr] = aux[lo:hi]
        in_maps.append({
            "yt": yt_c,
            "ypal": yp_c,
            "aux": ax_c.reshape(r_pad // G, G),
        })

    res = run_bass_kernel_spmd(nc, in_maps, list(range(N_CORES)))
    _last_results = res

    num = 0.0
    k = 0.0
    for i in range(N_CORES):
        p = res.results[i]["partials"]
        num += float(p[:, 0].sum(dtype=np.float64))
        k += float(p[:, 1].sum(dtype=np.float64))
    return np.float32(-num / k)


# revision 7
# speedup vs baseline: 1.6057x; 1.0318x over previous
"""Trainium2 Bass kernel for nn_CrossEntropyLoss_22419729285187.

Computes  -sum_{matched, non-BG true rows} dot(y_true[i,1:], y_pred[rank_i]) / count
sharded over 8 NeuronCores.

Strategy (per sharding hint): shard y_true rows (N) across the 8 cores.
The host performs the cheap key join (encode + searchsorted + cumsum) to
produce, per true row, the positionally-aligned y_pred row and a validity
mask; the device streams the full feature payload (y_true_features shard,
aligned y_pred features, mask) and does the fused multiply-reduce +
count, emitting per-partition partial sums. Host combines 8x[128,2]
partials into the final scalar.
"""

import os
import sys

for _p in ("/opt/trn_rl_repo", "/root/.axon_site/_ro/trn_rl_repo"):
    if os.path.isdir(_p) and _p not in sys.path:
        sys.path.append(_p)

import numpy as np

N_CORES = 8

# Device-side tiling: rows are laid out [tile t][partition p][group g];
# each of the 128 partitions owns G consecutive rows per tile.
PARTS = 128
G = 64  # rows per partition per tile (main segment)

_compiled = {}
_last_results = None


def _encode(idx):
    idx = idx.astype(np.int64)
    return ((idx[:, 0] * 1024 + idx[:, 1]) * 1024 + idx[:, 2]) * 1024 + idx[:, 3]


def _build_program(segments, c_pred):
    """Build + schedule the SPMD Tile program for one core shard.

    segments: list of (n_tiles, G) — the shard's rows are laid out
    [tile][partition][group] per segment, concatenated. Using a small
    trailing segment keeps zero-padding minimal while the main segment
    uses large (1MB) DMA tiles.
    """
    from concourse import bacc
    import concourse.mybir as mybir
    from concourse.tile import TileContext

    f32 = mybir.dt.float32
    r_pad = sum(nt * PARTS * g for nt, g in segments)
    total_tiles = sum(nt for nt, _ in segments)

    nc = bacc.Bacc("TRN2", target_bir_lowering=False, debug=False,
                   num_devices=N_CORES)
    yt_d = nc.dram_tensor("yt", [r_pad, c_pred], f32, kind="ExternalInput")
    yp_d = nc.dram_tensor("ypal", [r_pad, c_pred], f32, kind="ExternalInput")
    ax_d = nc.dram_tensor("aux", [r_pad, 1], f32, kind="ExternalInput")
    out_d = nc.dram_tensor("partials", [PARTS, 2], f32, kind="ExternalOutput")

    with TileContext(nc) as tc:
        with tc.tile_pool(name="acc", bufs=1) as accp:
            red_all = accp.tile([PARTS, total_tiles], f32)
            k_all = accp.tile([PARTS, total_tiles], f32)
            num_acc = accp.tile([PARTS, 1], f32)
            k_acc = accp.tile([PARTS, 1], f32)
            with tc.tile_pool(name="io", bufs=5) as pool, \
                 tc.tile_pool(name="scrp", bufs=2) as scrp:
                row0 = 0
                ti = 0
                for nt, g in segments:
                    seg_rows = nt * PARTS * g
                    yt_v = yt_d.ap()[row0:row0 + seg_rows, :].rearrange(
                        "(t p g) c -> t p (g c)", p=PARTS, g=g)
                    yp_v = yp_d.ap()[row0:row0 + seg_rows, :].rearrange(
                        "(t p g) c -> t p (g c)", p=PARTS, g=g)
                    ax_v = ax_d.ap()[row0:row0 + seg_rows, :].rearrange(
                        "(t p g) c -> t p (g c)", p=PARTS, g=g)
                    row0 += seg_rows
                    for t in range(nt):
                        yt_t = pool.tile([PARTS, g * c_pred], f32, tag="yt")
                        yp_t = pool.tile([PARTS, g * c_pred], f32, tag="yp")
                        ax_t = pool.tile([PARTS, g], f32, tag="ax")
                        nc.sync.dma_start(out=yt_t[:], in_=yt_v[t])
                        nc.scalar.dma_start(out=yp_t[:], in_=yp_v[t])
                        nc.gpsimd.dma_start(out=ax_t[:], in_=ax_v[t])
                        scr = scrp.tile([PARTS, g * c_pred], f32, tag="scr")
                        kscr = scrp.tile([PARTS, g], f32, tag="kscr")
                        # red_all[:, ti] = sum_{g,c} yt * ypal
                        nc.vector.scalar_tensor_tensor(
                            out=scr[:], in0=yt_t[:], scalar=1.0, in1=yp_t[:],
                            op0=mybir.AluOpType.mult, op1=mybir.AluOpType.mult,
                            accum_out=red_all[:, ti:ti + 1])
                        # k_all[:, ti] = sum_g mask
                        nc.vector.scalar_tensor_tensor(
                            out=kscr[:], in0=ax_t[:], scalar=1.0, in1=ax_t[:],
                            op0=mybir.AluOpType.mult, op1=mybir.AluOpType.mult,
                            accum_out=k_all[:, ti:ti + 1])
                        ti += 1
            nc.vector.tensor_reduce(out=num_acc[:], in_=red_all[:],
                                    axis=mybir.AxisListType.X,
                                    op=mybir.AluOpType.add)
            nc.vector.tensor_reduce(out=k_acc[:], in_=k_all[:],
                                    axis=mybir.AxisListType.X,
                                    op=mybir.AluOpType.add)
            nc.sync.dma_start(out=out_d[:, 0:1], in_=num_acc[:])
            nc.sync.dma_start(out=out_d[:, 1:2], in_=k_acc[:])
    nc.compile()
    return nc


def kernel(y_true_features, y_true_indices, y_pred_features, y_pred_indices):
    global _last_results
    from concourse.bass_utils import run_bass_kernel_spmd

    yt = np.ascontiguousarray(np.asarray(y_true_features, dtype=np.float32))
    yp = np.ascontiguousarray(np.asarray(y_pred_features, dtype=np.float32))
    n, c1 = yt.shape
    m, c = yp.shape

    # ---- host-side key join (cheap integer work) ----
    kt = _encode(np.asarray(y_true_indices))
    kp = _encode(np.asarray(y_pred_indices))
    kps = np.sort(kp)
    pos = np.clip(np.searchsorted(kps, kt), 0, m - 1)
    matched = kps[pos] == kt
    # Only matched true rows contribute to num and k. The r-th matched
    # true row (row order) pairs with y_pred_features[r] positionally
    # (rank = cumsum(matched)-1 is sequential over matched rows), so the
    # pred side needs no gather at all — just the first m_eff rows.
    midx = np.flatnonzero(matched)
    m_eff = midx.size
    yt_cmp = yt[midx, 1:]                      # [m_eff, c] gather
    notbg = yt[midx, 0] != 1.0
    yt_cmp[~notbg] = 0.0                       # BG pairs contribute 0
    aux = notbg.astype(np.float32)

    # ---- shard the m_eff matched pairs across cores ----
    rows = -(-m_eff // N_CORES)
    big = PARTS * G
    nt1 = rows // big
    rem = rows - nt1 * big
    g2 = -(-rem // PARTS)
    segments = ((nt1, G), (1, g2)) if g2 > 0 else ((nt1, G),)
    r_pad = sum(nt * PARTS * g for nt, g in segments)

    key = (segments, c)
    if key not in _compiled:
        _compiled[key] = _build_program(segments, c)
    nc = _compiled[key]

    in_maps = []
    for i in range(N_CORES):
        lo, hi = i * rows, min((i + 1) * rows, m_eff)
        nr = max(hi - lo, 0)
        yt_c = np.zeros((r_pad, c), dtype=np.float32)
        yt_c[:nr] = yt_cmp[lo:hi]
        yp_c = np.zeros((r_pad, c), dtype=np.float32)
        yp_c[:nr] = yp[lo:hi]
        ax_c = np.zeros((r_pad, 1), dtype=np.float32)
        ax_c[:nr, 0] = aux[lo:hi]
        in_maps.append({"yt": yt_c, "ypal": yp_c, "aux": ax_c})

    res = run_bass_kernel_spmd(nc, in_maps, list(range(N_CORES)))
    _last_results = res

    num = 0.0
    k = 0.0
    for i in range(N_CORES):
        p = res.results[i]["partials"]
        num += float(p[:, 0].sum(dtype=np.float64))
        k += float(p[:, 1].sum(dtype=np.float64))
    return np.float32(-num / k)
